# revision 35
# baseline (speedup 1.0000x reference)
"""Grouped GEMM (MoE expert-parallel) Trainium2 kernel.

Problem: Y_i = X_i @ W_i^T for 64 experts, X_i = 256 contiguous token rows of
input_tokens [16384, 2048] f32, W_i = weight_stack[i] [2048, 2048] f32.

Strategy (expert-parallel across 8 NeuronCores):
  - Core c owns experts [8c, 8c+8) and their 2048 tokens.
  - Host-side "shard" step also retransposes the operands so every device DMA
    is a fast contiguous load (the PE contracts over the partition dim, so both
    matmul operands need K on partitions; X and W are stored K-innermost).
    P-major layouts give one contiguous DRAM run per SBUF partition per DMA.
  - Device: for each expert, accumulate over K in PSUM (8 banks = 2 m-tiles
    x 4 n-blocks of 512), with W streamed from HBM under the matmuls.
  - Default mode "hy" (hybrid precision): per expert, k-chunks 0..13 run the
    "w8" path (X bf16 stationary x W e3m4 moving, 1 PE cyc/row) and chunks
    14,15 run as ONE fp8 DoubleRow matmul per output tile (both operands
    e4m3; perf_mode=DoubleRow contracts 2 k-chunks per instruction at
    2 fp8 MACs/PE/cycle) -> PE work drops from 16 to 15 instruction-
    equivalents per tile (218us -> 204.8us ideal; the PE is the roofline,
    DMA is ~50MB/core ~ 146us). All 16 chunks accumulate into the same
    PSUM fp32 group at product scale 64 (acc: X*1 @ 64W; fast: 2X @ 32W);
    the psum->sbuf copy applies the 1/64 descale. rel err =
    sqrt((14*1.34^2 + 2*3.75^2)/16) = 1.834e-2 < 2e-2 gate.
  - Start/tail trims: 5 zero-input warm-up matmuls spin the PE clock while
    the first (split, multi-queue) X/W transfers land; the last expert's
    fast-pair W is prefetched, its m=1 psum copies are halved across
    vector+scalar, and its Y stores issue per-512-block on sync/scalar so
    the final 512KB store isn't serialized on one queue.
  - Measured: 227.4us, rel err 1.834e-2 (vs 244.5us w8 baseline; pure-fp8
    x8 runs 158us but at 3.75e-2 fails the gate). Stream is PE-bound at
    ~215ns per 512-row matmul + 404ns/expert DoubleRow mode-switch tax.
  - Mode ladder (TRN_GG_MODE): fp32 917us exact, bf16x3 696us 4e-6,
    fp32r 513us 1.3e-4, bf16 ~300us 2e-3, w8 244.5us 1.35e-2,
    hy 227.4us 1.83e-2 (default), x8 158us 3.75e-2 (fails).
  - Output Y is produced in natural [token, n] layout bf16; host concatenates
    and upcasts.
"""

import os
import numpy as np

E_TOTAL = 64
K_FEAT = 2048
N_FEAT = 2048
TPE = 256                      # tokens per expert
NCORES = 8
EPC = E_TOTAL // NCORES        # experts per core
MC = EPC * TPE                 # tokens per core
P = 128
KC = K_FEAT // P               # 16 k-chunks
NB = N_FEAT // 512             # 4 n-blocks of 512
MT = TPE // P                  # 2 m-tiles per expert

MODE = os.environ.get("TRN_GG_MODE", "hy")   # fp32 | fp32r | bf16 | bf16x3 | w8 | x8 | hy
KCF = int(os.environ.get("TRN_GG_KCF", "2"))  # hy: fast (DoubleRow) k-chunks, even
KCA = KC - KCF                                # hy: accurate (w8-style) k-chunks

_compiled = {}
_last_exec_time_ns = None
_last_result = None


def _build(mode):
    import concourse.bacc as bacc
    import concourse.bass as bass
    import concourse.mybir as mybir
    from concourse import tile

    f32 = mybir.dt.float32
    bf16 = mybir.dt.bfloat16
    if mode in ("bf16", "bf16x3"):
        in_dt = bf16
    elif mode == "w8":
        in_dt = bf16                           # X dtype; W overridden below
    elif mode == "x8":
        in_dt = mybir.dt.float8e4              # both operands e4m3 (DoubleRow)
    elif mode == "fp32r":
        in_dt = mybir.dt.float32r
    else:
        in_dt = f32
    x_dt = in_dt
    w_dt = mybir.dt.float8e3 if mode == "w8" else in_dt
    nsplit = 2 if mode == "bf16x3" else 1     # hi/lo input streams
    dbl = mode == "x8"                         # fp8 DoubleRow: pair k-chunks
    descale = 1.0 / 64.0 if mode == "x8" else None

    nc = bacc.Bacc(None, target_bir_lowering=False)
    xts = []
    wts = []
    for s in range(nsplit):
        sfx = ("_hi", "_lo")[s] if nsplit == 2 else ""
        xts.append(nc.declare_dram_parameter(f"xt{sfx}", [EPC, P, KC, TPE], x_dt, isOutput=False))
        wts.append(nc.declare_dram_parameter(f"wt{sfx}", [EPC, P, KC, N_FEAT], w_dt, isOutput=False))
    y = None  # declared after out_dt is known (below)


    wch = int(os.environ.get("TRN_GG_WCH", "2"))
    wbufs = int(os.environ.get("TRN_GG_WBUFS", "12"))
    xbufs = int(os.environ.get("TRN_GG_XBUFS", "3"))
    obufs = int(os.environ.get("TRN_GG_OBUFS", "6"))
    y_eng = os.environ.get("TRN_GG_YENG", "gpsimd")
    y_bf16 = bool(int(os.environ.get("TRN_GG_YBF16", "1")))   # store Y as bf16
    alt_copy = bool(int(os.environ.get("TRN_GG_ALTCOPY", "1")))  # psum copies on 2 engines
    out_dt = bf16 if y_bf16 else f32
    w_alt = bool(int(os.environ.get("TRN_GG_WALT", "0")))
    ramp = bool(int(os.environ.get("TRN_GG_RAMP", "1")))
    split_x = bool(int(os.environ.get("TRN_GG_SPLITX", "1")))
    nb_stores = bool(int(os.environ.get("TRN_GG_NBSTORES", "0")))
    y = nc.declare_dram_parameter("y", [MC, N_FEAT], out_dt, isOutput=True)

    def chunk_plan(e):
        plan = [(c0, min(wch, KC - c0)) for c0 in range(0, KC, wch)]
        if dbl:
            # DoubleRow consumes k-chunks in aligned pairs; keep every
            # chunk even-sized and even-aligned (no ramp halving/taper).
            return plan
        if ramp and wch > 1:
            if e == 0:
                # halve only the first chunk so the first matmul's dependency
                # lands earlier without fragmenting the whole stream
                (c0, sz), rest = plan[0], plan[1:]
                plan = [(c0, sz // 2), (c0 + sz // 2, sz - sz // 2)] + rest
            if e == EPC - 1:
                # taper the last chunks so the PE drains within ~2us of the
                # final W byte instead of a full-chunk lag
                (c0, sz), head = plan[-1], plan[:-1]
                plan = head + [(c0 + i, 1) for i in range(sz)]
        return plan
    with tile.TileContext(nc) as tc:
        with (
            tc.tile_pool(name="xe", bufs=xbufs) as xpool,
            tc.tile_pool(name="w", bufs=wbufs) as wpool,
            tc.tile_pool(name="o", bufs=obufs) as opool,
            tc.tile_pool(name="ps", bufs=8, space=bass.MemorySpace.PSUM) as pspool,
        ):
            warm = int(os.environ.get("TRN_GG_WARM", "8"))
            if warm:
                # PE warm-up: zero-dependency matmuls at t=0 so the HAM clock
                # gate reaches K=8/8 (2.4 GHz) while the first W chunk is
                # still in flight. Shares the psum ring (write-only, never
                # read); wu source is memset so no uninitialized-read dep.
                wu = xpool.tile([P, 512], x_dt, tag="wu", name="wu", bufs=1)
                nc.vector.memset(wu[:], 0.0)
                wps = pspool.tile([P, 512], f32, tag="ps", name="wu_ps")
                for i in range(warm):
                    nc.tensor.matmul(
                        wps[:], wu[:, 0:P], wu[:],
                        start=(i == 0), stop=(i == warm - 1),
                    )
            for e in range(EPC):
                xe = [xpool.tile([P, KC, TPE], x_dt, tag=f"xe{s}", name=f"xe{s}_{e}") for s in range(nsplit)]
                for s in range(nsplit):
                    if split_x:
                        h = KC // 2
                        nc.scalar.dma_start(out=xe[s][:, :h, :], in_=xts[s][e, :, :h, :])
                        nc.scalar.dma_start(out=xe[s][:, h:, :], in_=xts[s][e, :, h:, :])
                    else:
                        nc.scalar.dma_start(out=xe[s][:], in_=xts[s][e])
                psums = [[pspool.tile([P, 512], f32, tag="ps", name=f"ps_{e}_{m}_{nb}") for nb in range(NB)] for m in range(MT)]
                for c0, csz in chunk_plan(e):
                    wtl = [wpool.tile([P, csz, N_FEAT], w_dt, tag=f"w{s}", name=f"w{s}_{e}_{c0}") for s in range(nsplit)]
                    for s in range(nsplit):
                        weng = nc.scalar if (w_alt and ((c0 // csz) % 2)) else nc.sync
                        weng.dma_start(out=wtl[s][:], in_=wts[s][e, :, c0:c0 + csz, :])
                    if dbl:
                        for j in range(0, csz, 2):
                            kc = c0 + j
                            for m in range(MT):
                                for nb in range(NB):
                                    nc.tensor.matmul(
                                        psums[m][nb][:],
                                        xe[0][:, kc:kc + 2, m * P:(m + 1) * P],
                                        wtl[0][:, j:j + 2, nb * 512:(nb + 1) * 512],
                                        start=(kc == 0),
                                        stop=(kc == KC - 2),
                                        perf_mode=mybir.MatmulPerfMode.DoubleRow,
                                    )
                        continue
                    for j in range(csz):
                        kc = c0 + j
                        # (x_s, w_s) passes accumulated per output tile
                        passes = [(0, 0)] if nsplit == 1 else [(0, 0), (1, 0), (0, 1)]
                        for m in range(MT):
                            for nb in range(NB):
                                for pi, (sx, sw) in enumerate(passes):
                                    nc.tensor.matmul(
                                        psums[m][nb][:],
                                        xe[sx][:, kc, m * P:(m + 1) * P],
                                        wtl[sw][:, j, nb * 512:(nb + 1) * 512],
                                        start=(kc == 0 and pi == 0),
                                        stop=(kc == KC - 1 and pi == len(passes) - 1),
                                    )
                for m in range(MT):
                    yt = opool.tile([P, N_FEAT], out_dt, tag="yt", name=f"yt_{e}_{m}")
                    for nb in range(NB):
                        dst = yt[:, nb * 512:(nb + 1) * 512]
                        if descale is not None:
                            if alt_copy and (nb % 2):
                                nc.scalar.mul(dst, psums[m][nb][:], descale)
                            else:
                                nc.vector.tensor_scalar_mul(dst, psums[m][nb][:], descale)
                        elif alt_copy and (nb % 2):
                            nc.scalar.copy(dst, psums[m][nb][:])
                        else:
                            nc.vector.tensor_copy(dst, psums[m][nb][:])
                        if nb_stores:
                            getattr(nc, y_eng).dma_start(
                                out=y[e * TPE + m * P:e * TPE + (m + 1) * P,
                                      nb * 512:(nb + 1) * 512],
                                in_=yt[:, nb * 512:(nb + 1) * 512],
                            )
                    if not nb_stores:
                        # tail stores go on the scalar HWDGE ring (X loads are
                        # done by then; SWDGE's ~1us/DMA fixed cost would
                        # serialize into the kernel tail) as one issue per
                        # m-tile: the Act sequencer's 667ns/dma_start is the
                        # tail critical path, so fewer issues beat finer grain
                        yeng = nc.scalar if e == EPC - 1 else getattr(nc, y_eng)
                        yeng.dma_start(
                            out=y[e * TPE + m * P:e * TPE + (m + 1) * P, :], in_=yt[:]
                        )
    nc.compile()
    return nc


def _build_hy():
    """Hybrid: per expert, KCA k-chunks in the w8 path (X bf16 x W e3m4,
    1 cyc/row) + KCF k-chunks as e4m3 DoubleRow pairs (2 fp8 MACs/PE/cyc),
    all accumulated in the same PSUM group at product scale 64 (acc: X*1 @
    64W; fast: 2X @ 32W), descaled 1/64 in the psum->sbuf copy.

    KCF=2 keeps rel err ~1.83e-2 (< 2e-2 gate): err^2 = (14*1.34^2 +
    2*3.75^2)/16. Tensor ideal drops 218us -> 204us.
    """
    import concourse.bacc as bacc
    import concourse.bass as bass
    import concourse.mybir as mybir
    from concourse import tile

    f32 = mybir.dt.float32
    bf16 = mybir.dt.bfloat16
    e3 = mybir.dt.float8e3
    e4 = mybir.dt.float8e4

    qtr = bool(int(os.environ.get("TRN_GG_QTR", "0")))

    nc = bacc.Bacc(None, target_bir_lowering=False)
    xa = nc.declare_dram_parameter("xa", [EPC, P, KCA, TPE], bf16, isOutput=False)
    xf = nc.declare_dram_parameter("xf", [EPC, P, KCF, TPE], e4, isOutput=False)
    wa = nc.declare_dram_parameter("wa", [EPC, P, KCA, N_FEAT], e3, isOutput=False)
    wf = nc.declare_dram_parameter("wf", [EPC, P, KCF, N_FEAT], e4, isOutput=False)
    if qtr:
        # chunks KCA-2,KCA-1 also run DoubleRow on the first 512 output
        # cols (w8 on the rest): +0.5 fast-chunk-equivalents for
        # sqrt((12*1.80+2*9.73+2*14.06)/16) ~ 1.93e-2 rel err
        xf2 = nc.declare_dram_parameter("xf2", [EPC, P, 2, TPE], e4, isOutput=False)
        wf2 = nc.declare_dram_parameter("wf2", [EPC, P, 2, 512], e4, isOutput=False)
    y = nc.declare_dram_parameter("y", [MC, N_FEAT], bf16, isOutput=True)

    wch = int(os.environ.get("TRN_GG_WCH", "2"))
    wbufs = int(os.environ.get("TRN_GG_WBUFS", "12"))
    xbufs = int(os.environ.get("TRN_GG_XBUFS", "3"))
    obufs = int(os.environ.get("TRN_GG_OBUFS", "6"))
    y_eng = os.environ.get("TRN_GG_YENG", "gpsimd")
    warm = int(os.environ.get("TRN_GG_WARM", "5"))
    w0split = bool(int(os.environ.get("TRN_GG_W0SPLIT", "1")))
    tailnb = bool(int(os.environ.get("TRN_GG_TAILNB", "1")))
    descale = 1.0 / 64.0

    def chunk_plan(e):
        plan = [(c0, min(wch, KCA - c0)) for c0 in range(0, KCA, wch)]
        if wch > 1:
            if e == 0:
                (c0, sz), rest = plan[0], plan[1:]
                if sz > 1:
                    plan = [(c0, sz // 2), (c0 + sz // 2, sz - sz // 2)] + rest
            if e == EPC - 1:
                (c0, sz), head = plan[-1], plan[:-1]
                plan = head + [(c0 + i, 1) for i in range(sz)]
        return plan

    with tile.TileContext(nc) as tc:
        with (
            tc.tile_pool(name="xe", bufs=xbufs) as xpool,
            tc.tile_pool(name="w", bufs=wbufs) as wpool,
            tc.tile_pool(name="wf", bufs=3) as wfpool,
            tc.tile_pool(name="o", bufs=obufs) as opool,
            tc.tile_pool(name="ps", bufs=8, space=bass.MemorySpace.PSUM) as pspool,
        ):
            if warm:
                wu = xpool.tile([P, 512], bf16, tag="wu", name="wu", bufs=1)
                nc.vector.memset(wu[:], 0.0)
                wps = pspool.tile([P, 512], f32, tag="ps", name="wu_ps")
                for i in range(warm):
                    nc.tensor.matmul(
                        wps[:], wu[:, 0:P], wu[:],
                        start=(i == 0), stop=(i == warm - 1),
                    )
            for e in range(EPC):
                xeA = xpool.tile([P, KCA, TPE], bf16, tag="xea", name=f"xea_{e}")
                if e == 0 and w0split:
                    # small first piece so the k=0 matmuls aren't gated on a
                    # half-MB X transfer
                    nc.scalar.dma_start(out=xeA[:, :2, :], in_=xa[e, :, :2, :])
                    nc.scalar.dma_start(out=xeA[:, 2:8, :], in_=xa[e, :, 2:8, :])
                    nc.scalar.dma_start(out=xeA[:, 8:, :], in_=xa[e, :, 8:, :])
                else:
                    h = KCA // 2
                    nc.scalar.dma_start(out=xeA[:, :h, :], in_=xa[e, :, :h, :])
                    nc.scalar.dma_start(out=xeA[:, h:, :], in_=xa[e, :, h:, :])
                xeF = xpool.tile([P, KCF, TPE], e4, tag="xef", name=f"xef_{e}")
                nc.scalar.dma_start(out=xeF[:], in_=xf[e])
                xeF2 = wtf2 = None
                if qtr:
                    xeF2 = xpool.tile([P, 2, TPE], e4, tag="xf2", name=f"xf2_{e}")
                    nc.scalar.dma_start(out=xeF2[:], in_=xf2[e])
                psums = [[pspool.tile([P, 512], f32, tag="ps", name=f"ps_{e}_{m}_{nb}") for nb in range(NB)] for m in range(MT)]
                wtf = None
                if e == EPC - 1:
                    # last expert: get the (stream-final) fast-pair W in
                    # flight before the acc chunks so the stop matmuls and
                    # the tail never wait on it
                    wtf = wfpool.tile([P, KCF, N_FEAT], e4, tag="wf", name=f"wf_{e}")
                    nc.sync.dma_start(out=wtf[:], in_=wf[e])
                    if qtr:
                        wtf2 = wfpool.tile([P, 2, 512], e4, tag="wf2", name=f"wf2_{e}")
                        nc.sync.dma_start(out=wtf2[:], in_=wf2[e])
                first_chunk = True
                for c0, csz in chunk_plan(e):
                    wtl = wpool.tile([P, csz, N_FEAT], e3, tag="w", name=f"w_{e}_{c0}")
                    if e == 0 and first_chunk and w0split:
                        # halve the very first W transfer across two queues so
                        # the first real matmul's data lands ~2us earlier
                        hn = N_FEAT // 2
                        nc.sync.dma_start(out=wtl[:, :, :hn], in_=wa[e, :, c0:c0 + csz, :hn])
                        nc.gpsimd.dma_start(out=wtl[:, :, hn:], in_=wa[e, :, c0:c0 + csz, hn:])
                    else:
                        nc.sync.dma_start(out=wtl[:], in_=wa[e, :, c0:c0 + csz, :])
                    first_chunk = False
                    for j in range(csz):
                        kc = c0 + j
                        for m in range(MT):
                            for nb in range(NB):
                                if qtr and kc >= KCA - 2 and nb == 0:
                                    continue  # covered by the qtr DR pair
                                nc.tensor.matmul(
                                    psums[m][nb][:],
                                    xeA[:, kc, m * P:(m + 1) * P],
                                    wtl[:, j, nb * 512:(nb + 1) * 512],
                                    start=(kc == 0),
                                    stop=False,
                                )
                if wtf is None:
                    wtf = wfpool.tile([P, KCF, N_FEAT], e4, tag="wf", name=f"wf_{e}")
                    nc.sync.dma_start(out=wtf[:], in_=wf[e])
                    if qtr:
                        wtf2 = wfpool.tile([P, 2, 512], e4, tag="wf2", name=f"wf2_{e}")
                        nc.sync.dma_start(out=wtf2[:], in_=wf2[e])
                if qtr:
                    # the qtr DR pair sits adjacent to the k14-15 DR block so
                    # the PE pays a single row-mode switch per expert
                    for m in range(MT):
                        nc.tensor.matmul(
                            psums[m][0][:],
                            xeF2[:, 0:2, m * P:(m + 1) * P],
                            wtf2[:, 0:2, :],
                            start=False,
                            stop=False,
                            perf_mode=mybir.MatmulPerfMode.DoubleRow,
                        )
                m_order = (
                    tuple(reversed(range(MT)))
                    if (e == EPC - 1 and tailm1) else tuple(range(MT))
                )
                for p0 in range(0, KCF, 2):
                    for m in m_order:
                        for nb in range(NB):
                            nc.tensor.matmul(
                                psums[m][nb][:],
                                xeF[:, p0:p0 + 2, m * P:(m + 1) * P],
                                wtf[:, p0:p0 + 2, nb * 512:(nb + 1) * 512],
                                start=False,
                                stop=(p0 == KCF - 2),
                                perf_mode=mybir.MatmulPerfMode.DoubleRow,
                            )
                for m in m_order:
                    yt = opool.tile([P, N_FEAT], bf16, tag="yt", name=f"yt_{e}_{m}")
                    row0 = e * TPE + m * P
                    tail = e == EPC - 1 and tailnb
                    last_tile = m == (m_order[-1] if tailm1 else MT - 1)
                    if tail and last_tile:
                        # the kernel's critical tail: halve each copy across
                        # both engines (parallel ~340ns instead of 680ns
                        # serial), then per-nb stores with issue order chosen
                        # so the store issues never delay a pending copy
                        for nb in range(NB):
                            ps = psums[m][nb][:]
                            d0 = yt[:, nb * 512:nb * 512 + 256]
                            d1 = yt[:, nb * 512 + 256:(nb + 1) * 512]
                            nc.vector.tensor_scalar_mul(d0, ps[:, :256], descale)
                            nc.scalar.mul(d1, ps[:, 256:], descale)
                        for nb in range(NB):
                            eng = nc.scalar if nb == NB - 1 else nc.sync
                            eng.dma_start(
                                out=y[row0:row0 + P, nb * 512:(nb + 1) * 512],
                                in_=yt[:, nb * 512:(nb + 1) * 512],
                            )
                        continue
                    if tail:
                        for nb in range(NB):
                            dst = yt[:, nb * 512:(nb + 1) * 512]
                            if nb % 2:
                                nc.scalar.mul(dst, psums[m][nb][:], descale)
                            else:
                                nc.vector.tensor_scalar_mul(dst, psums[m][nb][:], descale)
                        nc.sync.dma_start(out=y[row0:row0 + P, :], in_=yt[:])
                        continue
                    for nb in range(NB):
                        dst = yt[:, nb * 512:(nb + 1) * 512]
                        if nb % 2:
                            nc.scalar.mul(dst, psums[m][nb][:], descale)
                        else:
                            nc.vector.tensor_scalar_mul(dst, psums[m][nb][:], descale)
                    if False:
                        pass
                    else:
                        yeng = nc.scalar if e == EPC - 1 else getattr(nc, y_eng)
                        yeng.dma_start(out=y[row0:row0 + P, :], in_=yt[:])
    nc.compile()
    return nc


def _prep_inputs(input_tokens, weight_stack, mode):
    """Host-side shard + layout prep: per-core transposed, contiguous slices."""
    import ml_dtypes

    bf16 = ml_dtypes.bfloat16
    in_maps = []
    for c in range(NCORES):
        x_c = input_tokens[c * MC:(c + 1) * MC]                  # [MC, K]
        w_c = weight_stack[c * EPC:(c + 1) * EPC]                # [EPC, N, K]
        # P-major layouts: one contiguous DRAM run per SBUF partition per DMA
        xt_c = np.ascontiguousarray(
            x_c.reshape(EPC, TPE, KC, P).transpose(0, 3, 2, 1))  # [e, p, kc, m]
        wt_c = np.ascontiguousarray(
            w_c.reshape(EPC, N_FEAT, KC, P).transpose(0, 3, 2, 1))  # [e, p, kc, n]
        if mode == "bf16":
            in_maps.append({"xt": xt_c.astype(bf16), "wt": wt_c.astype(bf16)})
        elif mode == "x8":
            # Both operands e4m3 for DoubleRow (2 fp8 MACs/PE/cycle).
            # X*2 keeps small X out of the subnormal zone; W*32 centers W
            # (sigma 0.02) in range. Product scale 64 -> 1/64 on psum copy.
            e4 = ml_dtypes.float8_e4m3
            in_maps.append({
                "xt": np.clip(xt_c * 2.0, -240, 240).astype(e4),
                "wt": np.clip(wt_c * 32.0, -240, 240).astype(e4),
            })
        elif mode == "hy":
            e4 = ml_dtypes.float8_e4m3
            im = {
                "xa": np.ascontiguousarray(xt_c[:, :, :KCA, :]).astype(bf16),
                "xf": np.clip(
                    np.ascontiguousarray(xt_c[:, :, KCA:, :]) * 2.0, -240, 240
                ).astype(e4),
                "wa": np.clip(
                    np.ascontiguousarray(wt_c[:, :, :KCA, :]) * 64.0, -15.5, 15.5
                ).astype(ml_dtypes.float8_e3m4),
                "wf": np.clip(
                    np.ascontiguousarray(wt_c[:, :, KCA:, :]) * 32.0, -240, 240
                ).astype(e4),
            }
            if bool(int(os.environ.get("TRN_GG_QTR", "0"))):
                im["xf2"] = np.clip(
                    np.ascontiguousarray(xt_c[:, :, KCA - 2:KCA, :]) * 2.0,
                    -240, 240).astype(e4)
                im["wf2"] = np.clip(
                    np.ascontiguousarray(wt_c[:, :, KCA - 2:KCA, :512]) * 32.0,
                    -240, 240).astype(e4)
            in_maps.append(im)
        elif mode == "w8":
            # W in fp8-E3M4 (4 mantissa bits), X in bf16. Fold the fp8 range
            # scale s into X (power of two: exact in both formats), so
            # (X/s) @ (W*s)^T needs no descaling on device.
            s = 64.0
            xt8 = (xt_c / s).astype(bf16)
            wt8 = np.clip(wt_c * s, -15.5, 15.5).astype(ml_dtypes.float8_e3m4)
            in_maps.append({"xt": xt8, "wt": wt8})
        elif mode == "bf16x3":
            xt_hi = xt_c.astype(bf16)
            wt_hi = wt_c.astype(bf16)
            xt_lo = (xt_c - xt_hi.astype(np.float32)).astype(bf16)
            wt_lo = (wt_c - wt_hi.astype(np.float32)).astype(bf16)
            in_maps.append({"xt_hi": xt_hi, "xt_lo": xt_lo,
                            "wt_hi": wt_hi, "wt_lo": wt_lo})
        else:
            in_maps.append({"xt": xt_c, "wt": wt_c})
    return in_maps


def kernel(input_tokens, weight_stack, m_sizes, m_offsets):
    global _last_exec_time_ns, _last_result
    input_tokens = np.asarray(input_tokens, dtype=np.float32)
    weight_stack = np.asarray(weight_stack, dtype=np.float32)
    m_sizes = np.asarray(m_sizes)

    if not (m_sizes.shape == (E_TOTAL,) and np.all(m_sizes == TPE)):
        # General ragged fallback (not exercised by the fixed-shape harness).
        off = 0
        out = np.empty((input_tokens.shape[0], N_FEAT), np.float32)
        for i, sz in enumerate(m_sizes):
            sz = int(sz)
            out[off:off + sz] = input_tokens[off:off + sz] @ weight_stack[i].T
            off += sz
        return out

    from concourse.bass_utils import run_bass_kernel_spmd

    mode = MODE
    if mode not in _compiled:
        _compiled[mode] = _build_hy() if mode == "hy" else _build(mode)
    nc = _compiled[mode]

    in_maps = _prep_inputs(input_tokens, weight_stack, mode)
    trace = bool(int(os.environ.get("TRN_GG_TRACE", "0")))
    res = run_bass_kernel_spmd(nc, in_maps, core_ids=list(range(NCORES)), trace=trace)
    _last_exec_time_ns = res.exec_time_ns
    _last_result = res
    out = np.concatenate([res.results[c]["y"] for c in range(NCORES)], axis=0)
    return np.ascontiguousarray(out).astype(np.float32)



# revision 36
# speedup vs baseline: 1.1837x; 1.1837x over previous
"""Grouped GEMM (MoE expert-parallel) Trainium2 kernel.

Problem: Y_i = X_i @ W_i^T for 64 experts, X_i = 256 contiguous token rows of
input_tokens [16384, 2048] f32, W_i = weight_stack[i] [2048, 2048] f32.

Strategy (expert-parallel across 8 NeuronCores):
  - Core c owns experts [8c, 8c+8) and their 2048 tokens.
  - Host-side "shard" step also retransposes the operands so every device DMA
    is a fast contiguous load (the PE contracts over the partition dim, so both
    matmul operands need K on partitions; X and W are stored K-innermost).
    P-major layouts give one contiguous DRAM run per SBUF partition per DMA.
  - Device: for each expert, accumulate over K in PSUM (8 banks = 2 m-tiles
    x 4 n-blocks of 512), with W streamed from HBM under the matmuls.
  - Default mode "hy" (hybrid precision): per expert, k-chunks 0..13 run the
    "w8" path (X bf16 stationary x W e3m4 moving, 1 PE cyc/row) and chunks
    14,15 run as ONE fp8 DoubleRow matmul per output tile (both operands
    e4m3; perf_mode=DoubleRow contracts 2 k-chunks per instruction at
    2 fp8 MACs/PE/cycle) -> PE work drops from 16 to 15 instruction-
    equivalents per tile (218us -> 204.8us ideal; the PE is the roofline,
    DMA is ~50MB/core ~ 146us). All 16 chunks accumulate into the same
    PSUM fp32 group at product scale 64 (acc: X*1 @ 64W; fast: 2X @ 32W);
    the psum->sbuf copy applies the 1/64 descale. rel err =
    sqrt((14*1.34^2 + 2*3.75^2)/16) = 1.834e-2 < 2e-2 gate.
  - Start/tail trims: 5 zero-input warm-up matmuls spin the PE clock while
    the first (split, multi-queue) X/W transfers land; the last expert's
    fast-pair W is prefetched, its m=1 psum copies are halved across
    vector+scalar, and its Y stores issue per-512-block on sync/scalar so
    the final 512KB store isn't serialized on one queue.
  - Measured: 227.4us, rel err 1.834e-2 (vs 244.5us w8 baseline; pure-fp8
    x8 runs 158us but at 3.75e-2 fails the gate). Stream is PE-bound at
    ~215ns per 512-row matmul + 404ns/expert DoubleRow mode-switch tax.
  - Mode ladder (TRN_GG_MODE): fp32 917us exact, bf16x3 696us 4e-6,
    fp32r 513us 1.3e-4, bf16 ~300us 2e-3, w8 244.5us 1.35e-2,
    hy 227.4us 1.83e-2 (default), x8 158us 3.75e-2 (fails).
  - Output Y is produced in natural [token, n] layout bf16; host concatenates
    and upcasts.
"""

import os
import numpy as np

E_TOTAL = 64
K_FEAT = 2048
N_FEAT = 2048
TPE = 256                      # tokens per expert
NCORES = 8
EPC = E_TOTAL // NCORES        # experts per core
MC = EPC * TPE                 # tokens per core
P = 128
KC = K_FEAT // P               # 16 k-chunks
NB = N_FEAT // 512             # 4 n-blocks of 512
MT = TPE // P                  # 2 m-tiles per expert

MODE = os.environ.get("TRN_GG_MODE", "hy")   # fp32 | fp32r | bf16 | bf16x3 | w8 | x8 | hy
KCF = int(os.environ.get("TRN_GG_KCF", "2"))  # hy: fast (DoubleRow) k-chunks, even
KCA = KC - KCF                                # hy: accurate (w8-style) k-chunks

_compiled = {}
_last_exec_time_ns = None
_last_result = None


def _build(mode):
    import concourse.bacc as bacc
    import concourse.bass as bass
    import concourse.mybir as mybir
    from concourse import tile

    f32 = mybir.dt.float32
    bf16 = mybir.dt.bfloat16
    if mode in ("bf16", "bf16x3"):
        in_dt = bf16
    elif mode == "w8":
        in_dt = bf16                           # X dtype; W overridden below
    elif mode == "x8":
        in_dt = mybir.dt.float8e4              # both operands e4m3 (DoubleRow)
    elif mode == "fp32r":
        in_dt = mybir.dt.float32r
    else:
        in_dt = f32
    x_dt = in_dt
    w_dt = mybir.dt.float8e3 if mode == "w8" else in_dt
    nsplit = 2 if mode == "bf16x3" else 1     # hi/lo input streams
    dbl = mode == "x8"                         # fp8 DoubleRow: pair k-chunks
    descale = 1.0 / 64.0 if mode == "x8" else None

    nc = bacc.Bacc(None, target_bir_lowering=False)
    xts = []
    wts = []
    for s in range(nsplit):
        sfx = ("_hi", "_lo")[s] if nsplit == 2 else ""
        xts.append(nc.declare_dram_parameter(f"xt{sfx}", [EPC, P, KC, TPE], x_dt, isOutput=False))
        wts.append(nc.declare_dram_parameter(f"wt{sfx}", [EPC, P, KC, N_FEAT], w_dt, isOutput=False))
    y = None  # declared after out_dt is known (below)


    wch = int(os.environ.get("TRN_GG_WCH", "2"))
    wbufs = int(os.environ.get("TRN_GG_WBUFS", "12"))
    xbufs = int(os.environ.get("TRN_GG_XBUFS", "3"))
    obufs = int(os.environ.get("TRN_GG_OBUFS", "6"))
    y_eng = os.environ.get("TRN_GG_YENG", "gpsimd")
    y_bf16 = bool(int(os.environ.get("TRN_GG_YBF16", "1")))   # store Y as bf16
    alt_copy = bool(int(os.environ.get("TRN_GG_ALTCOPY", "1")))  # psum copies on 2 engines
    out_dt = bf16 if y_bf16 else f32
    w_alt = bool(int(os.environ.get("TRN_GG_WALT", "0")))
    ramp = bool(int(os.environ.get("TRN_GG_RAMP", "1")))
    split_x = bool(int(os.environ.get("TRN_GG_SPLITX", "1")))
    nb_stores = bool(int(os.environ.get("TRN_GG_NBSTORES", "0")))
    y = nc.declare_dram_parameter("y", [MC, N_FEAT], out_dt, isOutput=True)

    def chunk_plan(e):
        plan = [(c0, min(wch, KC - c0)) for c0 in range(0, KC, wch)]
        if dbl:
            # DoubleRow consumes k-chunks in aligned pairs; keep every
            # chunk even-sized and even-aligned (no ramp halving/taper).
            return plan
        if ramp and wch > 1:
            if e == 0:
                # halve only the first chunk so the first matmul's dependency
                # lands earlier without fragmenting the whole stream
                (c0, sz), rest = plan[0], plan[1:]
                plan = [(c0, sz // 2), (c0 + sz // 2, sz - sz // 2)] + rest
            if e == EPC - 1:
                # taper the last chunks so the PE drains within ~2us of the
                # final W byte instead of a full-chunk lag
                (c0, sz), head = plan[-1], plan[:-1]
                plan = head + [(c0 + i, 1) for i in range(sz)]
        return plan
    with tile.TileContext(nc) as tc:
        with (
            tc.tile_pool(name="xe", bufs=xbufs) as xpool,
            tc.tile_pool(name="w", bufs=wbufs) as wpool,
            tc.tile_pool(name="o", bufs=obufs) as opool,
            tc.tile_pool(name="ps", bufs=8, space=bass.MemorySpace.PSUM) as pspool,
        ):
            warm = int(os.environ.get("TRN_GG_WARM", "8"))
            if warm:
                # PE warm-up: zero-dependency matmuls at t=0 so the HAM clock
                # gate reaches K=8/8 (2.4 GHz) while the first W chunk is
                # still in flight. Shares the psum ring (write-only, never
                # read); wu source is memset so no uninitialized-read dep.
                wu = xpool.tile([P, 512], x_dt, tag="wu", name="wu", bufs=1)
                nc.vector.memset(wu[:], 0.0)
                wps = pspool.tile([P, 512], f32, tag="ps", name="wu_ps")
                for i in range(warm):
                    nc.tensor.matmul(
                        wps[:], wu[:, 0:P], wu[:],
                        start=(i == 0), stop=(i == warm - 1),
                    )
            for e in range(EPC):
                xe = [xpool.tile([P, KC, TPE], x_dt, tag=f"xe{s}", name=f"xe{s}_{e}") for s in range(nsplit)]
                for s in range(nsplit):
                    if split_x:
                        h = KC // 2
                        nc.scalar.dma_start(out=xe[s][:, :h, :], in_=xts[s][e, :, :h, :])
                        nc.scalar.dma_start(out=xe[s][:, h:, :], in_=xts[s][e, :, h:, :])
                    else:
                        nc.scalar.dma_start(out=xe[s][:], in_=xts[s][e])
                psums = [[pspool.tile([P, 512], f32, tag="ps", name=f"ps_{e}_{m}_{nb}") for nb in range(NB)] for m in range(MT)]
                for c0, csz in chunk_plan(e):
                    wtl = [wpool.tile([P, csz, N_FEAT], w_dt, tag=f"w{s}", name=f"w{s}_{e}_{c0}") for s in range(nsplit)]
                    for s in range(nsplit):
                        weng = nc.scalar if (w_alt and ((c0 // csz) % 2)) else nc.sync
                        weng.dma_start(out=wtl[s][:], in_=wts[s][e, :, c0:c0 + csz, :])
                    if dbl:
                        for j in range(0, csz, 2):
                            kc = c0 + j
                            for m in range(MT):
                                for nb in range(NB):
                                    nc.tensor.matmul(
                                        psums[m][nb][:],
                                        xe[0][:, kc:kc + 2, m * P:(m + 1) * P],
                                        wtl[0][:, j:j + 2, nb * 512:(nb + 1) * 512],
                                        start=(kc == 0),
                                        stop=(kc == KC - 2),
                                        perf_mode=mybir.MatmulPerfMode.DoubleRow,
                                    )
                        continue
                    for j in range(csz):
                        kc = c0 + j
                        # (x_s, w_s) passes accumulated per output tile
                        passes = [(0, 0)] if nsplit == 1 else [(0, 0), (1, 0), (0, 1)]
                        for m in range(MT):
                            for nb in range(NB):
                                for pi, (sx, sw) in enumerate(passes):
                                    nc.tensor.matmul(
                                        psums[m][nb][:],
                                        xe[sx][:, kc, m * P:(m + 1) * P],
                                        wtl[sw][:, j, nb * 512:(nb + 1) * 512],
                                        start=(kc == 0 and pi == 0),
                                        stop=(kc == KC - 1 and pi == len(passes) - 1),
                                    )
                for m in range(MT):
                    yt = opool.tile([P, N_FEAT], out_dt, tag="yt", name=f"yt_{e}_{m}")
                    for nb in range(NB):
                        dst = yt[:, nb * 512:(nb + 1) * 512]
                        if descale is not None:
                            if alt_copy and (nb % 2):
                                nc.scalar.mul(dst, psums[m][nb][:], descale)
                            else:
                                nc.vector.tensor_scalar_mul(dst, psums[m][nb][:], descale)
                        elif alt_copy and (nb % 2):
                            nc.scalar.copy(dst, psums[m][nb][:])
                        else:
                            nc.vector.tensor_copy(dst, psums[m][nb][:])
                        if nb_stores:
                            getattr(nc, y_eng).dma_start(
                                out=y[e * TPE + m * P:e * TPE + (m + 1) * P,
                                      nb * 512:(nb + 1) * 512],
                                in_=yt[:, nb * 512:(nb + 1) * 512],
                            )
                    if not nb_stores:
                        # tail stores go on the scalar HWDGE ring (X loads are
                        # done by then; SWDGE's ~1us/DMA fixed cost would
                        # serialize into the kernel tail) as one issue per
                        # m-tile: the Act sequencer's 667ns/dma_start is the
                        # tail critical path, so fewer issues beat finer grain
                        yeng = nc.scalar if e == EPC - 1 else getattr(nc, y_eng)
                        yeng.dma_start(
                            out=y[e * TPE + m * P:e * TPE + (m + 1) * P, :], in_=yt[:]
                        )
    nc.compile()
    return nc


def _build_hy():
    """Hybrid: per expert, KCA k-chunks in the w8 path (X bf16 x W e3m4,
    1 cyc/row) + KCF k-chunks as e4m3 DoubleRow pairs (2 fp8 MACs/PE/cyc),
    all accumulated in the same PSUM group at product scale 64 (acc: X*1 @
    64W; fast: 2X @ 32W), descaled 1/64 in the psum->sbuf copy.

    KCF=2 keeps rel err ~1.83e-2 (< 2e-2 gate): err^2 = (14*1.34^2 +
    2*3.75^2)/16. Tensor ideal drops 218us -> 204us.
    """
    import concourse.bacc as bacc
    import concourse.bass as bass
    import concourse.mybir as mybir
    from concourse import tile

    f32 = mybir.dt.float32
    bf16 = mybir.dt.bfloat16
    e3 = mybir.dt.float8e3
    e4 = mybir.dt.float8e4

    qtr = bool(int(os.environ.get("TRN_GG_QTR", "0")))

    nc = bacc.Bacc(None, target_bir_lowering=False)
    xa = nc.declare_dram_parameter("xa", [EPC, P, KCA, TPE], bf16, isOutput=False)
    xf = nc.declare_dram_parameter("xf", [EPC, P, KCF, TPE], e4, isOutput=False)
    wa = nc.declare_dram_parameter("wa", [EPC, P, KCA, N_FEAT], e3, isOutput=False)
    wf = nc.declare_dram_parameter("wf", [EPC, P, KCF, N_FEAT], e4, isOutput=False)
    if qtr:
        # chunks KCA-2,KCA-1 also run DoubleRow on the first 512 output
        # cols (w8 on the rest): +0.5 fast-chunk-equivalents for
        # sqrt((12*1.80+2*9.73+2*14.06)/16) ~ 1.93e-2 rel err
        xf2 = nc.declare_dram_parameter("xf2", [EPC, P, 2, TPE], e4, isOutput=False)
        wf2 = nc.declare_dram_parameter("wf2", [EPC, P, 2, 512], e4, isOutput=False)
    y = nc.declare_dram_parameter("y", [MC, N_FEAT], bf16, isOutput=True)

    wch = int(os.environ.get("TRN_GG_WCH", "2"))
    wbufs = int(os.environ.get("TRN_GG_WBUFS", "12"))
    xbufs = int(os.environ.get("TRN_GG_XBUFS", "3"))
    obufs = int(os.environ.get("TRN_GG_OBUFS", "6"))
    y_eng = os.environ.get("TRN_GG_YENG", "gpsimd")
    warm = int(os.environ.get("TRN_GG_WARM", "5"))
    w0split = bool(int(os.environ.get("TRN_GG_W0SPLIT", "1")))
    tailnb = bool(int(os.environ.get("TRN_GG_TAILNB", "1")))
    tailm1 = bool(int(os.environ.get("TRN_GG_TAILM1", "0")))
    descale = 1.0 / 64.0

    def chunk_plan(e):
        plan = [(c0, min(wch, KCA - c0)) for c0 in range(0, KCA, wch)]
        if wch > 1:
            if e == 0:
                (c0, sz), rest = plan[0], plan[1:]
                if sz > 1:
                    plan = [(c0, sz // 2), (c0 + sz // 2, sz - sz // 2)] + rest
            if e == EPC - 1:
                (c0, sz), head = plan[-1], plan[:-1]
                plan = head + [(c0 + i, 1) for i in range(sz)]
        return plan

    with tile.TileContext(nc) as tc:
        with (
            tc.tile_pool(name="xe", bufs=xbufs) as xpool,
            tc.tile_pool(name="w", bufs=wbufs) as wpool,
            tc.tile_pool(name="wf", bufs=3) as wfpool,
            tc.tile_pool(name="o", bufs=obufs) as opool,
            tc.tile_pool(name="ps", bufs=8, space=bass.MemorySpace.PSUM) as pspool,
        ):
            if warm:
                wu = xpool.tile([P, 512], bf16, tag="wu", name="wu", bufs=1)
                nc.vector.memset(wu[:], 0.0)
                wps = pspool.tile([P, 512], f32, tag="ps", name="wu_ps")
                for i in range(warm):
                    nc.tensor.matmul(
                        wps[:], wu[:, 0:P], wu[:],
                        start=(i == 0), stop=(i == warm - 1),
                    )
            for e in range(EPC):
                xeA = xpool.tile([P, KCA, TPE], bf16, tag="xea", name=f"xea_{e}")
                if e == 0 and w0split:
                    # small first piece so the k=0 matmuls aren't gated on a
                    # half-MB X transfer
                    nc.scalar.dma_start(out=xeA[:, :2, :], in_=xa[e, :, :2, :])
                    nc.scalar.dma_start(out=xeA[:, 2:8, :], in_=xa[e, :, 2:8, :])
                    nc.scalar.dma_start(out=xeA[:, 8:, :], in_=xa[e, :, 8:, :])
                else:
                    h = KCA // 2
                    nc.scalar.dma_start(out=xeA[:, :h, :], in_=xa[e, :, :h, :])
                    nc.scalar.dma_start(out=xeA[:, h:, :], in_=xa[e, :, h:, :])
                xeF = xpool.tile([P, KCF, TPE], e4, tag="xef", name=f"xef_{e}")
                nc.scalar.dma_start(out=xeF[:], in_=xf[e])
                xeF2 = wtf2 = None
                if qtr:
                    xeF2 = xpool.tile([P, 2, TPE], e4, tag="xf2", name=f"xf2_{e}")
                    nc.scalar.dma_start(out=xeF2[:], in_=xf2[e])
                psums = [[pspool.tile([P, 512], f32, tag="ps", name=f"ps_{e}_{m}_{nb}") for nb in range(NB)] for m in range(MT)]
                wtf = None
                if e == EPC - 1:
                    # last expert: get the (stream-final) fast-pair W in
                    # flight before the acc chunks so the stop matmuls and
                    # the tail never wait on it
                    wtf = wfpool.tile([P, KCF, N_FEAT], e4, tag="wf", name=f"wf_{e}")
                    nc.sync.dma_start(out=wtf[:], in_=wf[e])
                    if qtr:
                        wtf2 = wfpool.tile([P, 2, 512], e4, tag="wf2", name=f"wf2_{e}")
                        nc.sync.dma_start(out=wtf2[:], in_=wf2[e])
                first_chunk = True
                for c0, csz in chunk_plan(e):
                    wtl = wpool.tile([P, csz, N_FEAT], e3, tag="w", name=f"w_{e}_{c0}")
                    if e == 0 and first_chunk and w0split:
                        # halve the very first W transfer across two queues so
                        # the first real matmul's data lands ~2us earlier
                        hn = N_FEAT // 2
                        nc.sync.dma_start(out=wtl[:, :, :hn], in_=wa[e, :, c0:c0 + csz, :hn])
                        nc.gpsimd.dma_start(out=wtl[:, :, hn:], in_=wa[e, :, c0:c0 + csz, hn:])
                    else:
                        nc.sync.dma_start(out=wtl[:], in_=wa[e, :, c0:c0 + csz, :])
                    first_chunk = False
                    for j in range(csz):
                        kc = c0 + j
                        for m in range(MT):
                            for nb in range(NB):
                                if qtr and kc >= KCA - 2 and nb == 0:
                                    continue  # covered by the qtr DR pair
                                nc.tensor.matmul(
                                    psums[m][nb][:],
                                    xeA[:, kc, m * P:(m + 1) * P],
                                    wtl[:, j, nb * 512:(nb + 1) * 512],
                                    start=(kc == 0),
                                    stop=False,
                                )
                if wtf is None:
                    wtf = wfpool.tile([P, KCF, N_FEAT], e4, tag="wf", name=f"wf_{e}")
                    nc.sync.dma_start(out=wtf[:], in_=wf[e])
                    if qtr:
                        wtf2 = wfpool.tile([P, 2, 512], e4, tag="wf2", name=f"wf2_{e}")
                        nc.sync.dma_start(out=wtf2[:], in_=wf2[e])
                if qtr:
                    # the qtr DR pair sits adjacent to the k14-15 DR block so
                    # the PE pays a single row-mode switch per expert
                    for m in range(MT):
                        nc.tensor.matmul(
                            psums[m][0][:],
                            xeF2[:, 0:2, m * P:(m + 1) * P],
                            wtf2[:, 0:2, :],
                            start=False,
                            stop=False,
                            perf_mode=mybir.MatmulPerfMode.DoubleRow,
                        )
                m_order = (
                    tuple(reversed(range(MT)))
                    if (e == EPC - 1 and tailm1) else tuple(range(MT))
                )
                for p0 in range(0, KCF, 2):
                    for m in m_order:
                        for nb in range(NB):
                            nc.tensor.matmul(
                                psums[m][nb][:],
                                xeF[:, p0:p0 + 2, m * P:(m + 1) * P],
                                wtf[:, p0:p0 + 2, nb * 512:(nb + 1) * 512],
                                start=False,
                                stop=(p0 == KCF - 2),
                                perf_mode=mybir.MatmulPerfMode.DoubleRow,
                            )
                for m in m_order:
                    yt = opool.tile([P, N_FEAT], bf16, tag="yt", name=f"yt_{e}_{m}")
                    row0 = e * TPE + m * P
                    tail = e == EPC - 1 and tailnb
                    last_tile = m == (m_order[-1] if tailm1 else MT - 1)
                    if tail and last_tile:
                        # the kernel's critical tail: halve each copy across
                        # both engines (parallel ~340ns instead of 680ns
                        # serial), then per-nb stores with issue order chosen
                        # so the store issues never delay a pending copy
                        for nb in range(NB):
                            ps = psums[m][nb][:]
                            d0 = yt[:, nb * 512:nb * 512 + 256]
                            d1 = yt[:, nb * 512 + 256:(nb + 1) * 512]
                            nc.vector.tensor_scalar_mul(d0, ps[:, :256], descale)
                            nc.scalar.mul(d1, ps[:, 256:], descale)
                        for nb in range(NB):
                            eng = nc.scalar if nb == NB - 1 else nc.sync
                            eng.dma_start(
                                out=y[row0:row0 + P, nb * 512:(nb + 1) * 512],
                                in_=yt[:, nb * 512:(nb + 1) * 512],
                            )
                        continue
                    if tail:
                        for nb in range(NB):
                            dst = yt[:, nb * 512:(nb + 1) * 512]
                            if nb % 2:
                                nc.scalar.mul(dst, psums[m][nb][:], descale)
                            else:
                                nc.vector.tensor_scalar_mul(dst, psums[m][nb][:], descale)
                        nc.sync.dma_start(out=y[row0:row0 + P, :], in_=yt[:])
                        continue
                    for nb in range(NB):
                        dst = yt[:, nb * 512:(nb + 1) * 512]
                        if nb % 2:
                            nc.scalar.mul(dst, psums[m][nb][:], descale)
                        else:
                            nc.vector.tensor_scalar_mul(dst, psums[m][nb][:], descale)
                    if False:
                        pass
                    else:
                        yeng = nc.scalar if e == EPC - 1 else getattr(nc, y_eng)
                        yeng.dma_start(out=y[row0:row0 + P, :], in_=yt[:])
    nc.compile()
    return nc


def _prep_inputs(input_tokens, weight_stack, mode):
    """Host-side shard + layout prep: per-core transposed, contiguous slices."""
    import ml_dtypes

    bf16 = ml_dtypes.bfloat16
    in_maps = []
    for c in range(NCORES):
        x_c = input_tokens[c * MC:(c + 1) * MC]                  # [MC, K]
        w_c = weight_stack[c * EPC:(c + 1) * EPC]                # [EPC, N, K]
        # P-major layouts: one contiguous DRAM run per SBUF partition per DMA
        xt_c = np.ascontiguousarray(
            x_c.reshape(EPC, TPE, KC, P).transpose(0, 3, 2, 1))  # [e, p, kc, m]
        wt_c = np.ascontiguousarray(
            w_c.reshape(EPC, N_FEAT, KC, P).transpose(0, 3, 2, 1))  # [e, p, kc, n]
        if mode == "bf16":
            in_maps.append({"xt": xt_c.astype(bf16), "wt": wt_c.astype(bf16)})
        elif mode == "x8":
            # Both operands e4m3 for DoubleRow (2 fp8 MACs/PE/cycle).
            # X*2 keeps small X out of the subnormal zone; W*32 centers W
            # (sigma 0.02) in range. Product scale 64 -> 1/64 on psum copy.
            e4 = ml_dtypes.float8_e4m3
            in_maps.append({
                "xt": np.clip(xt_c * 2.0, -240, 240).astype(e4),
                "wt": np.clip(wt_c * 32.0, -240, 240).astype(e4),
            })
        elif mode == "hy":
            e4 = ml_dtypes.float8_e4m3
            im = {
                "xa": np.ascontiguousarray(xt_c[:, :, :KCA, :]).astype(bf16),
                "xf": np.clip(
                    np.ascontiguousarray(xt_c[:, :, KCA:, :]) * 2.0, -240, 240
                ).astype(e4),
                "wa": np.clip(
                    np.ascontiguousarray(wt_c[:, :, :KCA, :]) * 64.0, -15.5, 15.5
                ).astype(ml_dtypes.float8_e3m4),
                "wf": np.clip(
                    np.ascontiguousarray(wt_c[:, :, KCA:, :]) * 32.0, -240, 240
                ).astype(e4),
            }
            if bool(int(os.environ.get("TRN_GG_QTR", "0"))):
                im["xf2"] = np.clip(
                    np.ascontiguousarray(xt_c[:, :, KCA - 2:KCA, :]) * 2.0,
                    -240, 240).astype(e4)
                im["wf2"] = np.clip(
                    np.ascontiguousarray(wt_c[:, :, KCA - 2:KCA, :512]) * 32.0,
                    -240, 240).astype(e4)
            in_maps.append(im)
        elif mode == "w8":
            # W in fp8-E3M4 (4 mantissa bits), X in bf16. Fold the fp8 range
            # scale s into X (power of two: exact in both formats), so
            # (X/s) @ (W*s)^T needs no descaling on device.
            s = 64.0
            xt8 = (xt_c / s).astype(bf16)
            wt8 = np.clip(wt_c * s, -15.5, 15.5).astype(ml_dtypes.float8_e3m4)
            in_maps.append({"xt": xt8, "wt": wt8})
        elif mode == "bf16x3":
            xt_hi = xt_c.astype(bf16)
            wt_hi = wt_c.astype(bf16)
            xt_lo = (xt_c - xt_hi.astype(np.float32)).astype(bf16)
            wt_lo = (wt_c - wt_hi.astype(np.float32)).astype(bf16)
            in_maps.append({"xt_hi": xt_hi, "xt_lo": xt_lo,
                            "wt_hi": wt_hi, "wt_lo": wt_lo})
        else:
            in_maps.append({"xt": xt_c, "wt": wt_c})
    return in_maps


def kernel(input_tokens, weight_stack, m_sizes, m_offsets):
    global _last_exec_time_ns, _last_result
    input_tokens = np.asarray(input_tokens, dtype=np.float32)
    weight_stack = np.asarray(weight_stack, dtype=np.float32)
    m_sizes = np.asarray(m_sizes)

    if not (m_sizes.shape == (E_TOTAL,) and np.all(m_sizes == TPE)):
        # General ragged fallback (not exercised by the fixed-shape harness).
        off = 0
        out = np.empty((input_tokens.shape[0], N_FEAT), np.float32)
        for i, sz in enumerate(m_sizes):
            sz = int(sz)
            out[off:off + sz] = input_tokens[off:off + sz] @ weight_stack[i].T
            off += sz
        return out

    from concourse.bass_utils import run_bass_kernel_spmd

    mode = MODE
    if mode not in _compiled:
        _compiled[mode] = _build_hy() if mode == "hy" else _build(mode)
    nc = _compiled[mode]

    in_maps = _prep_inputs(input_tokens, weight_stack, mode)
    trace = bool(int(os.environ.get("TRN_GG_TRACE", "0")))
    res = run_bass_kernel_spmd(nc, in_maps, core_ids=list(range(NCORES)), trace=trace)
    _last_exec_time_ns = res.exec_time_ns
    _last_result = res
    out = np.concatenate([res.results[c]["y"] for c in range(NCORES)], axis=0)
    return np.ascontiguousarray(out).astype(np.float32)

